# revision 60
# baseline (speedup 1.0000x reference)
"""Trainium2 Bass kernel for nn_GAT_81209241633571 (2-layer GAT, 4 heads).

Strategy (8 NeuronCores, SPMD):
  - Edges (plus self-loops) are sorted by destination and sharded by
    destination-node range: core c owns 49 tiles of 128 nodes (50176 padded
    nodes total = 8*49*128). All edges into a core's nodes are processed by
    that core, so segment-softmax and aggregation need no cross-core
    reduction.
  - Layer-1 node tables (xs = x@W + per-node attention dots) are SHARDED:
    each core computes its own 49 tiles (7 groups of 7), writes them to
    local HBM, and a chunked AllGather (one per group, contiguous thanks to
    a chunk-major table layout) shares the full table while compute runs.
    Row layout (768B, bf16-typed): 512 fp8 feats (bitcast) | 4 bf16 as |
    pad. fp8 features cost ~0.9e-2 rel err (total 1.19e-2 < 2e-2 gate).
    Tables stay bf16-typed because fp8-typed gather/AllGather corrupts
    data; fp8 is only ever seen by SBUF-side bitcast views.
  - Edge blocks of 128 gather source rows with the custom dma_gather
    instruction (4 swdge queues, runs of <=8 blocks). The edge-loop phases
    avoid DVE copy/cast/tensor_scalar ops entirely: those enter 2-port
    perf mode, lock GpSimd out of SBUF, and starve SWDGE descriptor
    generation (gathers crawl at 7.4us instead of ~3us per 1024 rows).
    All elementwise work is tensor_tensor (never contends) or scalar-engine.
  - Scatter (segment sum) is a one-hot matmul: host-built fp8 one-hot
    matrices (edge->dst-slot) as the PE stationary operand accumulate the
    weighted feature sums and softmax denominators in PSUM (a matmul out
    is capped at 512 f32 = one PSUM bank, so den is a second matmul).
    Destination-side attention values are expanded edge-wise with the
    transposed one-hot as stationary. In phase 5 the forward one-hot is
    built on-device (tensor_tensor is_equal vs an iota row) instead of
    streamed.
  - Softmax uses exp without max subtraction (logits are O(10), safe in
    f32), and the division by the denominator happens per destination node
    after aggregation (fused with the outer LeakyReLU via Act.Prelu+scale).
  - The layer-2 node table is computed in the phase-2 tile tail via PE
    transposes, with the layer-2 attention dots folded into the same 256B
    row; its AllGather is chunked (one per 7 tiles) so it overlaps phase 2,
    and the dst-side ad2 column is harvested into SBUF at tile close so
    phase 5 never waits on the collective for it.
  - int16 gather indices only span 32768 rows, so edges are grouped into
    blocks whose (chunk-major-mapped) source rows are all < 32768 ("lo")
    or >= 32768 ("hi"); hi blocks gather from a base offset of 32768 rows.

The schedule (block counts per tile) is derived from the runtime edge data
and made uniform across cores by padding, so one NEFF serves all 8 cores.
Measured: 2626060 ns (session start) -> 1710469 ns, rel err 1.193e-2.
"""
import os
import sys
import numpy as np
import ml_dtypes

sys.path.insert(0, "/opt/trn_rl_repo")

import concourse.bass as bass
import concourse.bacc as bacc
import concourse.mybir as mybir
from concourse.tile import TileContext
from concourse.bass_utils import run_bass_kernel_spmd
from concourse.library_config import mlp

bf = ml_dtypes.bfloat16
f8 = ml_dtypes.float8_e4m3

N = 50000
E = 800000
F = 128
H = 4
C = 64
NEG = 0.2
ALPHA = 0.2
P = 128
NCORES = 8
HI = 32768
SENT = 255
RUNCAP = 8       # max blocks per dma_gather call (>1024 idx crashes HW Q7)
OCH = 32         # one-hot stream chunk (blocks per DMA)
G1 = 7           # phase-1 tiles per DMA group (49 own tiles = 7 groups)

EW = 384                  # XTAB row elems bf16 (768B): 512 fp8 feats (256
                          # bf16 slots, bitcast) + 4 bf16 'as' + 124 pad.
                          # Table stays bf16-typed so DMA/gather/AllGather
                          # never see fp8; SBUF views bitcast to fp8.
ASO = 256                 # as offset in XTAB row (bf16 elems)
FW = H * F                # fp8 feature width / xpd rhs width (512)

dt = mybir.dt
Alu = mybir.AluOpType
Act = mybir.ActivationFunctionType


# ---------------------------------------------------------------------------
# Host preprocessing
# ---------------------------------------------------------------------------

def preprocess(edge, n=N, ncores=NCORES):
    """Sort/shard/pad edges; build gather-index and one-hot streams.

    Returns cfg dict with the static schedule and per-core input arrays.
    """
    npad = -(-n * 1 // (P * ncores)) * (P * ncores)
    while npad < n:
        npad += P * ncores
    nt_all = npad // P
    nt_core = nt_all // ncores

    # self-loops are handled analytically at tile close (a per-node row
    # scale), not in the gather/scatter edge stream
    src = np.asarray(edge[0], np.int64)
    dst = np.asarray(edge[1], np.int64)
    # chunk-major gather-table layout (so chunked AllGathers have
    # contiguous outputs): node v -> row q*(ncores*CH) + c*CH + w with
    # c = v // (nt_core*P), z = v % (nt_core*P), q = z // CH, w = z % CH
    CH = G1 * P
    zc = src % (npad // ncores)
    src = ((zc // CH) * (ncores * CH) + (src // (npad // ncores)) * CH
           + zc % CH)
    order = np.argsort(dst, kind="stable")
    src, dst = src[order], dst[order]
    tile_of = dst // P
    core_of = tile_of // nt_core

    # bucket per (core, local tile, half)
    buckets = {}
    for c in range(ncores):
        m = core_of == c
        s_c, d_c, t_c = src[m], dst[m], tile_of[m]
        for j in range(nt_core):
            mm = t_c == c * nt_core + j
            s_t, d_t = s_c[mm], d_c[mm]
            lo = s_t < HI
            buckets[(c, j, 0)] = (s_t[lo], d_t[lo])
            buckets[(c, j, 1)] = (s_t[~lo], d_t[~lo])

    B = np.zeros((nt_core, 2), np.int64)
    for j in range(nt_core):
        for hf in (0, 1):
            mx = max(len(buckets[(c, j, hf)][0]) for c in range(ncores))
            B[j, hf] = -(-mx // P)

    # block stream: per tile, halves ordered by parity for gather-run merging
    halves_of = [(0, 1) if j % 2 == 0 else (1, 0) for j in range(nt_core)]
    tob, bhalf = [], []
    tile_first, tile_last = {}, {}
    for j in range(nt_core):
        nb_t = int(B[j, 0] + B[j, 1])
        if nb_t == 0:
            continue
        tile_first[j] = len(tob)
        for hf in halves_of[j]:
            for _ in range(int(B[j, hf])):
                tob.append(j)
                bhalf.append(hf)
        tile_last[j] = len(tob) - 1
    NB = len(tob)
    tob = np.array(tob, np.int64)
    bhalf = np.array(bhalf, np.int64)

    # gather runs: maximal same-half block runs, capped
    runs = []
    b = 0
    while b < NB:
        e_ = b
        while e_ < NB and bhalf[e_] == bhalf[b] and e_ - b < RUNCAP:
            e_ += 1
        runs.append((b, e_, int(bhalf[b])))
        b = e_

    # per-core streams
    gidx = np.zeros((ncores, NB * P), np.int16)
    dloc = np.full((ncores, NB * P), SENT, np.uint8)
    for c in range(ncores):
        pos = {}
        b = 0
        for j in range(nt_core):
            for hf in halves_of[j]:
                if B[j, hf]:
                    pos[(j, hf)] = b
                    b += int(B[j, hf])
        for j in range(nt_core):
            for hf in (0, 1):
                if not B[j, hf]:
                    continue
                s_t, d_t = buckets[(c, j, hf)]
                k = len(s_t)
                base = pos[(j, hf)] * P
                gidx[c, base:base + k] = (s_t % HI).astype(np.int16)
                tbase = (c * nt_core + j) * P
                dloc[c, base:base + k] = (d_t - tbase).astype(np.uint8)

    # idx layout for dma_gather: [128, NB*128/16] i16, idx i at (g*16 + i%16,
    # i//16) for all 8 groups g
    gidx_t = np.zeros((ncores, 128, NB * P // 16), np.int16)
    for c in range(ncores):
        w = gidx[c].reshape(-1, 16).T    # [16, NB*8]
        gidx_t[c] = np.tile(w, (8, 1))

    # one-hot streams fp8: O[e, b*128+d], OT[d, b*128+e]
    O8 = np.zeros((ncores, 128, NB * P), f8)
    OT8 = np.zeros((ncores, 128, NB * P), f8)
    ar = np.arange(P)
    for c in range(ncores):
        dl = dloc[c].reshape(NB, P)
        oh = (dl[:, :, None] == ar[None, None, :])    # [NB, e, d]
        O8[c] = np.ascontiguousarray(
            oh.transpose(1, 0, 2).reshape(P, NB * P)).astype(f8)
        OT8[c] = np.ascontiguousarray(
            oh.transpose(2, 0, 1).reshape(P, NB * P)).astype(f8)

    dlb = np.zeros((ncores, P, NB), bf)
    for c in range(ncores):
        dlb[c] = dloc[c].reshape(NB, P).T.astype(np.float32).astype(bf)

    return dict(n=n, npad=npad, nt_all=nt_all, nt_core=nt_core,
                ncores=ncores, NB=NB, runs=runs, tob=tob,
                tile_first=tile_first, tile_last=tile_last,
                gidx_t=gidx_t, O8=O8, OT8=OT8, dlb=dlb)


# ---------------------------------------------------------------------------
# Device program
# ---------------------------------------------------------------------------

def build(cfg, has_bias, has_bias2, phases=5):
    npad, nt_all, nt_core = cfg["npad"], cfg["nt_all"], cfg["nt_core"]
    ncores, NB = cfg["ncores"], cfg["NB"]
    runs, tob = cfg["runs"], cfg["tob"]
    tile_first, tile_last = cfg["tile_first"], cfg["tile_last"]

    nc = bacc.Bacc("TRN2", num_devices=ncores, enable_partition_id=True,
               num_swdge_queues=4)
    rg = [list(range(ncores))]

    # inputs
    xT_d = nc.dram_tensor("xT", [nt_core // G1, P, G1 * H * F], dt.bfloat16,
                      kind="ExternalInput")
    W_d = nc.dram_tensor("Wb", [H, F, F], dt.bfloat16, kind="ExternalInput")
    WT_d = nc.dram_tensor("WTb", [H, F, F], dt.bfloat16, kind="ExternalInput")
    ac_d = nc.dram_tensor("acol", [H, F, 2], dt.bfloat16, kind="ExternalInput")
    W2_d = nc.dram_tensor("W2b", [4, F, C], dt.bfloat16, kind="ExternalInput")
    W2T_d = nc.dram_tensor("W2Tb", [4, C, F], dt.bfloat16, kind="ExternalInput")
    a2_d = nc.dram_tensor("a2col", [C, 2], dt.bfloat16, kind="ExternalInput")
    gi_d = nc.dram_tensor("gidx", [P, NB * P // 16], dt.int16, kind="ExternalInput")
    O8_d = nc.dram_tensor("O8", [P, NB * P], dt.float8e4, kind="ExternalInput")
    OT8_d = nc.dram_tensor("OT8", [P, NB * P], dt.float8e4, kind="ExternalInput")
    dlb_d = nc.dram_tensor("dlb", [P, NB], dt.bfloat16, kind="ExternalInput")
    iota_d = nc.dram_tensor("iotab", [P, P], dt.bfloat16, kind="ExternalInput")
    id_d = nc.dram_tensor("ident", [P, P], dt.bfloat16, kind="ExternalInput")
    b1_d = nc.dram_tensor("b1rep", [H, P, F], dt.float32, kind="ExternalInput")
    b2_d = nc.dram_tensor("b2rep", [P, C], dt.float32, kind="ExternalInput")
    out_d = nc.dram_tensor("out", [nt_core * P, C], dt.float32, kind="ExternalOutput")

    # internal DRAM
    XT_SH = nc.dram_tensor("XT_SH", [nt_core * P, EW], dt.bfloat16,
                           kind="Internal")
    XTAB = nc.dram_tensor("XTAB", [npad, EW], dt.bfloat16, kind="Internal",
                          addr_space="Shared")
    X2SH = nc.dram_tensor("X2SH", [nt_core * P, 128], dt.bfloat16,
                          kind="Internal")
    X2G = nc.dram_tensor("X2G", [npad, 128], dt.bfloat16,
                         kind="Internal", addr_space="Shared")

    with TileContext(nc) as tc:
        nc.gpsimd.load_library(mlp)

        # ---------------- phase 0: weights prep ----------------
        import contextlib
        with tc.tile_pool(name="wsb", bufs=1) as wsb, \
             contextlib.ExitStack() as wps_stack:
            wps = wps_stack.enter_context(
                tc.tile_pool(name="wps", bufs=1, space="PSUM"))
            wrhs = []
            for h in range(H):
                wt = wsb.tile([F, F], dt.bfloat16, tag=f"wt{h}")
                nc.sync.dma_start(out=wt[:], in_=WT_d[h])
                acs = wsb.tile([F, 2], dt.bfloat16, tag=f"ac{h}")
                nc.sync.dma_start(out=acs[:], in_=ac_d[h])
                pw = wps.tile([F, 2], dt.float32, tag="pw")
                nc.tensor.matmul(out=pw[:], lhsT=wt[:], rhs=acs[:],
                                 start=True, stop=True)
                wr = wsb.tile([F, F + 2], dt.bfloat16, tag=f"wr{h}")
                nc.sync.dma_start(out=wr[:, 0:F], in_=W_d[h])
                nc.vector.tensor_copy(out=wr[:, F:F + 2], in_=pw[:])
                wrhs.append(wr)
            w2rhs = []
            for k in range(4):
                wt2 = wsb.tile([C, F], dt.bfloat16, tag="wt2")
                nc.sync.dma_start(out=wt2[:], in_=W2T_d[k])
                ac2 = wsb.tile([C, 2], dt.bfloat16, tag="ac2")
                nc.sync.dma_start(out=ac2[:], in_=a2_d[:])
                pw2 = wps.tile([F, 2], dt.float32, tag="pw")
                nc.tensor.matmul(out=pw2[:], lhsT=wt2[:], rhs=ac2[:],
                                 start=True, stop=True)
                w2 = wsb.tile([F, C + 2], dt.bfloat16, tag=f"w2r{k}")
                nc.sync.dma_start(out=w2[:, 0:C], in_=W2_d[k])
                nc.vector.tensor_copy(out=w2[:, C:C + 2], in_=pw2[:])
                w2rhs.append(w2)
            ident = wsb.tile([P, P], dt.bfloat16, tag="ident")
            nc.sync.dma_start(out=ident[:], in_=id_d[:])
            if has_bias:
                b1s = []
                for h in range(H):
                    t = wsb.tile([P, F], dt.float32, tag=f"b1_{h}")
                    nc.sync.dma_start(out=t[:], in_=b1_d[h])
                    b1s.append(t)
            if has_bias2:
                b2s = wsb.tile([P, C], dt.float32, tag="b2")
                nc.sync.dma_start(out=b2s[:], in_=b2_d[:])

            # gather indices resident
            gidx_sb = wsb.tile([P, NB * P // 16], dt.int16, tag="gi")
            nc.sync.dma_start(out=gidx_sb[:], in_=gi_d[:])
            # per-block dst-slot columns + iota rows for on-device one-hot gen
            dlb_sb = wsb.tile([P, NB], dt.bfloat16, tag="dlb")
            nc.sync.dma_start(out=dlb_sb[:], in_=dlb_d[:])
            iota_sb = wsb.tile([P, P], dt.bfloat16, tag="iota")
            nc.sync.dma_start(out=iota_sb[:], in_=iota_d[:])

            # always write out once so the output is defined even when
            # later phases are disabled
            zo = wsb.tile([P, C], dt.float32, tag="zo")
            nc.gpsimd.memset(zo[:], 0)
            nc.sync.dma_start(out=out_d[0:P, :], in_=zo[:])
            # leaky-relu slope as a per-partition AP (Act.Lrelu ignores its
            # alpha on TRN2 HW; Act.Prelu with an AP alpha works)
            al02 = wsb.tile([P, 1], dt.float32, tag="al02")
            nc.gpsimd.memset(al02[:], NEG)
            # epsilon tile for den clamping via tensor_tensor max (DVE
            # tensor_scalar enters 2-port perf mode and starves SWDGE
            # descriptor generation on gpsimd; tensor_tensor never does)
            epsb = wsb.tile([P, 4], dt.float32, tag="epsb")
            nc.gpsimd.memset(epsb[:], 1e-30)
            wps_stack.close()   # free the weights-prep PSUM bank

            # ------------- phase 1: L1 tables (sharded + AllGather) -------------
            ngrp = nt_core // G1
            adt_loc = wsb.tile([P, nt_core, 4], dt.bfloat16, tag="adtl")
            # own-tile L1/L2 rows kept resident for the analytic self-loop
            # term applied at tile close
            xown = wsb.tile([P, nt_core, EW], dt.bfloat16, tag="xown")
            x2own = wsb.tile([P, nt_core, C + 2], dt.bfloat16, tag="x2own")
            with tc.tile_pool(name="t1", bufs=3) as t1, \
                 tc.tile_pool(name="p1", bufs=6, space="PSUM") as p1:
                for g in range(ngrp if phases >= 1 else 0):
                    xt = t1.tile([P, G1, H * F], dt.bfloat16, tag="xt")
                    nc.sync.dma_start(out=xt[:], in_=xT_d[g])
                    xrow = t1.tile([P, G1, EW], dt.bfloat16, tag="xrow")
                    for t in range(G1):
                        gt = g * G1 + t
                        for hp in range(2):
                            ph = p1.tile([P, 2, F + 2], dt.float32, tag="ph")
                            for hh in range(2):
                                h = hp * 2 + hh
                                nc.tensor.matmul(
                                    out=ph[:, hh],
                                    lhsT=xt[:, t, h * F:(h + 1) * F],
                                    rhs=wrhs[h][:], start=True, stop=True)
                            fdst = xrow[:, t, hp * F:(hp + 1) * F].bitcast(
                                dt.float8e4)
                            if has_bias:
                                for hh in range(2):
                                    h = hp * 2 + hh
                                    nc.vector.tensor_tensor(
                                        out=xrow[:, t, h * F:(h + 1) * F],
                                        in0=ph[:, hh, 0:F], in1=b1s[h][:],
                                        op=Alu.add)
                            elif hp == 0:
                                nc.vector.tensor_copy(out=fdst, in_=ph[:, :, 0:F])
                            else:
                                nc.scalar.copy(out=fdst, in_=ph[:, :, 0:F])
                            # as values: bf16 packed into the fp8 row bytes
                            asv = xrow[:, t, ASO:ASO + 4]
                            if hp == 0:
                                nc.vector.tensor_copy(
                                    out=asv[:, 0:2],
                                    in_=ph[:, :, F:F + 1])
                                nc.vector.tensor_copy(
                                    out=adt_loc[:, gt, 0:2],
                                    in_=ph[:, :, F + 1:F + 2])
                            else:
                                nc.scalar.copy(
                                    out=asv[:, 2:4],
                                    in_=ph[:, :, F:F + 1])
                                nc.scalar.copy(
                                    out=adt_loc[:, gt, 2:4],
                                    in_=ph[:, :, F + 1:F + 2])
                    nc.vector.tensor_copy(
                        out=xown[:, g * G1:(g + 1) * G1, :], in_=xrow[:])
                    nc.scalar.dma_start(
                        out=XT_SH[bass.ds(g * G1 * P, G1 * P)].rearrange(
                            "(t p) c -> p t c", p=P), in_=xrow[:])
                    # per-group AllGather: chunk-major XTAB keeps each
                    # chunk's output contiguous; transfers overlap compute
                    CH = G1 * P
                    nc.gpsimd.collective_compute(
                        "AllGather", Alu.bypass, replica_groups=rg,
                        ins=[XT_SH[g * CH:(g + 1) * CH, :]],
                        outs=[XTAB[g * ncores * CH:(g + 1) * ncores * CH, :]])

            # ---------------- phase 2: L1 edge loop ----------------

            with tc.tile_pool(name="g2", bufs=7) as g2, \
                 tc.tile_pool(name="o2", bufs=3) as o2, \
                 tc.tile_pool(name="s2", bufs=8) as s2, \
                 tc.tile_pool(name="e2", bufs=6) as e2, \
                 tc.tile_pool(name="pp", bufs=2, space="PSUM") as pp, \
                 tc.tile_pool(name="pa", bufs=2, space="PSUM") as pa, \
                 tc.tile_pool(name="pd", bufs=2, space="PSUM") as pd, \
                 tc.tile_pool(name="ptp", bufs=1, space="PSUM") as ptp, \
                 tc.tile_pool(name="p2p", bufs=1, space="PSUM") as p2p:
                for _w in range(30):
                    pwm = p2p.tile([P, C + 2], dt.float32, tag="ps2")
                    nc.tensor.matmul(out=pwm[:], lhsT=ident[:],
                                     rhs=ident[:, 0:C + 2], start=True,
                                     stop=True)
                nch = -(-NB // OCH)
                o_t = [None] * nch
                ot_t = [None] * nch
                ps_out = ps_den = None
                for ri, (b0, b1, hf) in enumerate(
                        runs if phases >= 2 else []):
                    nb = b1 - b0
                    xg = g2.tile([P, nb, EW], dt.bfloat16, tag="xg")
                    tab = XTAB[HI:, :] if hf else XTAB[:, :]
                    nc.gpsimd.dma_gather(
                        xg[:], tab, gidx_sb[:, b0 * 8:b1 * 8],
                        nb * P, nb * P, EW, queue_num=ri % 4)
                    # one-hot chunks: prefetch up to 2 runs ahead
                    bpre = min(b1 + 2 * RUNCAP, NB)
                    for b in range(b0, bpre):
                        ch = b // OCH
                        if o_t[ch] is None:
                            cw = min(OCH * P, NB * P - ch * OCH * P)
                            ot = o2.tile([P, OCH * P], dt.float8e4, tag="oc")
                            nc.sync.dma_start(
                                out=ot[:, 0:cw],
                                in_=O8_d[:, ch * OCH * P:ch * OCH * P + cw])
                            ott = o2.tile([P, OCH * P], dt.float8e4, tag="otc")
                            nc.sync.dma_start(
                                out=ott[:, 0:cw],
                                in_=OT8_d[:, ch * OCH * P:ch * OCH * P + cw])
                            o_t[ch], ot_t[ch] = ot, ott
                    # batched dst-attention expansion for the whole run
                    pads = pa.tile([P, nb, 4], dt.float32, tag="pe")
                    for b in range(b0, b1):
                        k = b - b0
                        ch, coff = b // OCH, b % OCH
                        OTsl = ot_t[ch][:, coff * P:(coff + 1) * P]
                        nc.tensor.matmul(out=pads[:, k], lhsT=OTsl,
                                         rhs=adt_loc[:, int(tob[b])],
                                         start=True, stop=True)
                    e4 = e2.tile([P, nb, 4], dt.float32, tag="e4")
                    nc.vector.tensor_tensor(
                        out=e4[:],
                        in0=xg[:, :, ASO:ASO + 4],
                        in1=pads[:], op=Alu.add)
                    el = e2.tile([P, nb, 4], dt.float32, tag="el")
                    nc.scalar.activation(el[:], e4[:], Act.Prelu, alpha=al02[:, 0:1])
                    # fused per-run rhs: [feats*p4 | p4] (den cols at FW..)
                    xpd = s2.tile([P, nb, FW + 8], dt.bfloat16, tag="xpd",
                                  bufs=3)
                    nc.scalar.activation(xpd[:, :, FW:FW + 4], el[:], Act.Exp)
                    # per-block 3-d TTs first (fast DVE mode), then the
                    # matmuls: keeps PE streaming without per-block bubbles
                    for b in range(b0, b1):
                        k = b - b0
                        scb = xpd[:, k, FW:FW + 4].rearrange(
                            "p (h one) -> p h one", one=1).to_broadcast(
                            [P, H, F])
                        nc.vector.tensor_tensor(
                            out=xpd[:, k, 0:FW].rearrange(
                                "p (h f) -> p h f", h=H),
                            in0=xg[:, k, 0:ASO].bitcast(dt.float8e4).rearrange(
                                "p (h f) -> p h f", h=H),
                            in1=scb, op=Alu.mult)
                    for b in range(b0, b1):
                        k = b - b0
                        ch, coff = b // OCH, b % OCH
                        j = int(tob[b])
                        Osl = o_t[ch][:, coff * P:(coff + 1) * P]
                        first = b == tile_first[j]
                        last = b == tile_last[j]
                        if first:
                            ps_out = pp.tile([P, H * F], dt.float32, tag="po")
                            ps_den = pd.tile([P, 4], dt.float32, tag="pd")
                        nc.tensor.matmul(out=ps_out[:], lhsT=Osl,
                                         rhs=xpd[:, k, 0:FW],
                                         start=first, stop=last)
                        nc.tensor.matmul(out=ps_den[:], lhsT=Osl,
                                         rhs=xpd[:, k, FW:FW + 4],
                                         start=first, stop=last)
                        if last:
                            # analytic self-loop term: w=exp(lrelu(as+ad))
                            # per own node, an elementwise row scale (no PE)
                            es = e2.tile([P, 4], dt.float32, tag="es")
                            nc.vector.tensor_tensor(
                                out=es[:], in0=xown[:, j, ASO:ASO + 4],
                                in1=adt_loc[:, j], op=Alu.add)
                            els = e2.tile([P, 4], dt.float32, tag="els")
                            nc.scalar.activation(els[:], es[:], Act.Prelu,
                                                 alpha=al02[:, 0:1])
                            ws = e2.tile([P, 4], dt.bfloat16, tag="ws")
                            nc.scalar.activation(ws[:], els[:], Act.Exp)
                            wxs = s2.tile([P, H * F], dt.bfloat16, tag="wxs",
                                          bufs=2)
                            nc.vector.tensor_tensor(
                                out=wxs[:].rearrange("p (h f) -> p h f", h=H),
                                in0=xown[:, j, 0:ASO].bitcast(
                                    dt.float8e4).rearrange(
                                    "p (h f) -> p h f", h=H),
                                in1=ws[:].rearrange(
                                    "p (h one) -> p h one", one=1
                                ).to_broadcast([P, H, F]), op=Alu.mult)
                            sm = s2.tile([P, H * F], dt.float32, tag="sm",
                                         bufs=2)
                            nc.vector.tensor_tensor(
                                out=sm[:], in0=ps_out[:], in1=wxs[:],
                                op=Alu.add)
                            dsum = e2.tile([P, 4], dt.float32, tag="dsum")
                            nc.vector.tensor_tensor(
                                out=dsum[:], in0=ps_den[:], in1=ws[:],
                                op=Alu.add)
                            dg = e2.tile([P, 4], dt.float32, tag="dg")
                            nc.vector.tensor_tensor(
                                out=dg[:], in0=dsum[:], in1=epsb[:],
                                op=Alu.max)
                            rc = e2.tile([P, 4], dt.float32, tag="rc")
                            nc.vector.reciprocal(out=rc[:], in_=dg[:])
                            xl1 = s2.tile([P, H * F], dt.bfloat16, tag="xl1")
                            for h in range(H):
                                if has_bias:
                                    y0 = s2.tile([P, F], dt.float32, tag="y0")
                                    nc.scalar.activation(
                                        y0[:], sm[:, h * F:(h + 1) * F],
                                        Act.Copy, scale=rc[:, h:h + 1])
                                    y1 = s2.tile([P, F], dt.float32, tag="y1")
                                    nc.vector.tensor_tensor(
                                        out=y1[:], in0=y0[:], in1=b1s[h][:],
                                        op=Alu.add)
                                    nc.vector.scalar_tensor_tensor(
                                        out=xl1[:, h * F:(h + 1) * F],
                                        in0=y1[:], scalar=ALPHA, in1=y1[:],
                                        op0=Alu.mult, op1=Alu.max)
                                else:
                                    nc.scalar.activation(
                                        xl1[:, h * F:(h + 1) * F],
                                        sm[:, h * F:(h + 1) * F],
                                        Act.Prelu, scale=rc[:, h:h + 1],
                                        alpha=al02[:, 0:1])
                            # fused L2 table row: transpose xl1, matmul W2
                            ptr = ptp.tile([P, H * F], dt.bfloat16, tag="ptr")
                            for h in range(H):
                                nc.tensor.transpose(
                                    ptr[:, h * F:(h + 1) * F],
                                    xl1[:, h * F:(h + 1) * F], ident[:])
                            xl1T = s2.tile([P, H * F], dt.bfloat16, tag="xl1T")
                            nc.scalar.copy(out=xl1T[:, 0:2 * F],
                                           in_=ptr[:, 0:2 * F])
                            nc.scalar.copy(out=xl1T[:, 2 * F:4 * F],
                                           in_=ptr[:, 2 * F:4 * F])
                            ps2 = p2p.tile([P, C + 2], dt.float32, tag="ps2")
                            for q in range(4):
                                nc.tensor.matmul(
                                    out=ps2[:], lhsT=xl1T[:, q * F:(q + 1) * F],
                                    rhs=w2rhs[q][:], start=(q == 0),
                                    stop=(q == 3))
                            x2row = s2.tile([P, 128], dt.bfloat16, tag="x2row")
                            if has_bias2:
                                nc.vector.tensor_tensor(
                                    out=x2row[:, 0:C], in0=ps2[:, 0:C],
                                    in1=b2s[:], op=Alu.add)
                                nc.vector.tensor_copy(
                                    out=x2row[:, C:C + 2], in_=ps2[:, C:C + 2])
                            else:
                                nc.scalar.copy(out=x2row[:, 0:C + 2],
                                               in_=ps2[:])
                            nc.scalar.copy(out=x2own[:, j],
                                           in_=x2row[:, 0:C + 2])
                            nc.scalar.dma_start(
                                out=X2SH[j * P:(j + 1) * P, :], in_=x2row[:])
                            # chunked X2 AllGather: overlap ph2's tail
                            if phases >= 4 and j % G1 == G1 - 1:
                                q = j // G1
                                CH = G1 * P
                                nc.gpsimd.collective_compute(
                                    "AllGather", Alu.bypass,
                                    replica_groups=rg,
                                    ins=[X2SH[q * CH:(q + 1) * CH, :]],
                                    outs=[X2G[q * ncores * CH:
                                              (q + 1) * ncores * CH, :]])

            # ---------------- phase 4: AllGather (chunked, in ph2) ----------

            # ---------------- phase 5: L2 edge loop ----------------
            with tc.tile_pool(name="g5", bufs=8) as g5, \
                 tc.tile_pool(name="o5", bufs=3) as o5, \
                 tc.tile_pool(name="o5g", bufs=4) as o5g, \
                 tc.tile_pool(name="s5", bufs=8) as s5, \
                 tc.tile_pool(name="e5", bufs=6) as e5, \
                 tc.tile_pool(name="pq", bufs=2, space="PSUM") as pq, \
                 tc.tile_pool(name="pb", bufs=2, space="PSUM") as pb:
                for _w in range(30):
                    pwm = pq.tile([P, C + 1], dt.float32, tag="p5o")
                    nc.tensor.matmul(out=pwm[:], lhsT=ident[:],
                                     rhs=ident[:, 0:C + 1], start=True,
                                     stop=True)
                nch = -(-NB // OCH)
                ot_t = [None] * nch
                ps2o = None
                for ri, (b0, b1, hf) in enumerate(
                        runs if phases >= 5 else []):
                    nb = b1 - b0
                    xg2 = g5.tile([P, nb, 128], dt.bfloat16, tag="xg5")
                    tab = X2G[HI:, :] if hf else X2G[:, :]
                    nc.gpsimd.dma_gather(
                        xg2[:], tab, gidx_sb[:, b0 * 8:b1 * 8],
                        nb * P, nb * P, 128, queue_num=ri % 4)
                    bpre = min(b1 + 2 * RUNCAP, NB)
                    for b in range(b0, bpre):
                        ch = b // OCH
                        if ot_t[ch] is None:
                            cw = min(OCH * P, NB * P - ch * OCH * P)
                            ott = o5.tile([P, OCH * P], dt.float8e4, tag="otc5")
                            nc.sync.dma_start(
                                out=ott[:, 0:cw],
                                in_=OT8_d[:, ch * OCH * P:ch * OCH * P + cw])
                            ot_t[ch] = ott
                    pa2 = pb.tile([P, nb, 1], dt.float32, tag="p5e")
                    for b in range(b0, b1):
                        k = b - b0
                        ch, coff = b // OCH, b % OCH
                        OTsl = ot_t[ch][:, coff * P:(coff + 1) * P]
                        nc.tensor.matmul(out=pa2[:, k], lhsT=OTsl,
                                         rhs=x2own[:, int(tob[b]), 65:66],
                                         start=True, stop=True)
                    e1 = e5.tile([P, nb, 1], dt.float32, tag="e1")
                    nc.vector.tensor_tensor(
                        out=e1[:], in0=xg2[:, :, C:C + 1], in1=pa2[:],
                        op=Alu.add)
                    el1 = e5.tile([P, nb, 1], dt.float32, tag="el1")
                    nc.scalar.activation(el1[:], e1[:], Act.Prelu, alpha=al02[:, 0:1])
                    xpden = s5.tile([P, nb, C + 1], dt.bfloat16, tag="xpden")
                    nc.scalar.activation(xpden[:, :, C:C + 1], el1[:], Act.Exp)
                    nc.vector.tensor_tensor(
                        out=xpden[:, :, 0:C], in0=xg2[:, :, 0:C],
                        in1=xpden[:, :, C:C + 1].to_broadcast([P, nb, C]),
                        op=Alu.mult)
                    for b in range(b0, b1):
                        k = b - b0
                        j = int(tob[b])
                        # one-hot built on-device: O[e,d] = (dloc[e] == d)
                        og = o5g.tile([P, P], dt.bfloat16, tag="og")
                        nc.vector.tensor_tensor(
                            out=og[:],
                            in0=dlb_sb[:, b:b + 1].to_broadcast([P, P]),
                            in1=iota_sb[:], op=Alu.is_equal)
                        first = b == tile_first[j]
                        last = b == tile_last[j]
                        if first:
                            ps2o = pq.tile([P, C + 1], dt.float32, tag="p5o")
                        nc.tensor.matmul(out=ps2o[:], lhsT=og[:],
                                         rhs=xpden[:, k], start=first,
                                         stop=last)
                        if last:
                            # analytic self-loop term for layer 2
                            es2 = e5.tile([P, 1], dt.float32, tag="es2")
                            nc.vector.tensor_tensor(
                                out=es2[:], in0=x2own[:, j, C:C + 1],
                                in1=x2own[:, j, C + 1:C + 2], op=Alu.add)
                            els2 = e5.tile([P, 1], dt.float32, tag="els2")
                            nc.scalar.activation(els2[:], es2[:], Act.Prelu,
                                                 alpha=al02[:, 0:1])
                            ws2 = e5.tile([P, 1], dt.bfloat16, tag="ws2")
                            nc.scalar.activation(ws2[:], els2[:], Act.Exp)
                            wx2 = s5.tile([P, C], dt.bfloat16, tag="wx2",
                                          bufs=2)
                            nc.vector.tensor_tensor(
                                out=wx2[:], in0=x2own[:, j, 0:C],
                                in1=ws2[:].to_broadcast([P, C]), op=Alu.mult)
                            sm2 = s5.tile([P, C], dt.float32, tag="sm2",
                                          bufs=2)
                            nc.vector.tensor_tensor(
                                out=sm2[:], in0=ps2o[:, 0:C], in1=wx2[:],
                                op=Alu.add)
                            d2 = e5.tile([P, 1], dt.float32, tag="d2")
                            nc.vector.tensor_tensor(
                                out=d2[:], in0=ps2o[:, C:C + 1], in1=ws2[:],
                                op=Alu.add)
                            dg = e5.tile([P, 1], dt.float32, tag="dg5")
                            nc.vector.tensor_tensor(
                                out=dg[:], in0=d2[:],
                                in1=epsb[:, 0:1], op=Alu.max)
                            rc = e5.tile([P, 1], dt.float32, tag="rc5")
                            nc.vector.reciprocal(out=rc[:], in_=dg[:])
                            y = s5.tile([P, C], dt.float32, tag="y5")
                            if has_bias2:
                                nc.scalar.activation(
                                    y[:], sm2[:], Act.Copy,
                                    scale=rc[:, 0:1])
                                y1 = s5.tile([P, C], dt.float32, tag="y51")
                                nc.vector.tensor_tensor(
                                    out=y1[:], in0=y[:], in1=b2s[:],
                                    op=Alu.add)
                                nc.vector.scalar_tensor_tensor(
                                    out=y[:], in0=y1[:], scalar=ALPHA,
                                    in1=y1[:], op0=Alu.mult, op1=Alu.max)
                            else:
                                nc.scalar.activation(
                                    y[:], sm2[:], Act.Prelu,
                                    scale=rc[:, 0:1], alpha=al02[:, 0:1])
                            yo = s5.tile([P, C], dt.float32, tag="yo")
                            nc.scalar.activation(yo[:], y[:], Act.Tanh)
                            nc.scalar.dma_start(
                                out=out_d[j * P:(j + 1) * P, :], in_=yo[:])

    nc.compile()
    return nc


# ---------------------------------------------------------------------------
# Entry point
# ---------------------------------------------------------------------------

_CACHE = {}


def _inputs_for_core(cfg, c, inputs):
    type_emb = np.asarray(inputs["type_emb"], np.float32)
    W = np.asarray(inputs["W"], np.float32)
    a_src = np.asarray(inputs["att_src"], np.float32)
    a_dst = np.asarray(inputs["att_dst"], np.float32)
    W_out = np.asarray(inputs["W_out"], np.float32)
    a2s = np.asarray(inputs["att_src_out"], np.float32)
    a2d = np.asarray(inputs["att_dst_out"], np.float32)
    bias = np.asarray(inputs["bias"], np.float32)
    bias2 = np.asarray(inputs["bias_out"], np.float32)
    npad, nt_all, nt_core = cfg["npad"], cfg["nt_all"], cfg["nt_core"]
    n = cfg["n"]

    # xT tiles for this core's own range: [nt_core, 128fi, H*128n]
    xT = np.zeros((nt_core, P, H * F), bf)
    te = np.zeros((H, npad, F), np.float32)
    te[:, :n] = type_emb
    for j in range(nt_core):
        t = c * nt_core + j
        blk = te[:, t * P:(t + 1) * P, :]          # [H, n128, F]
        xT[j] = np.concatenate([blk[h].T for h in range(H)], axis=1).astype(bf)
    ngrp = nt_core // G1
    xT = np.ascontiguousarray(
        xT.reshape(ngrp, G1, P, H * F).transpose(0, 2, 1, 3).reshape(
            ngrp, P, G1 * H * F))
    acol = np.stack([np.stack([a_src[h], a_dst[h]], 1) for h in range(H)])
    W2k = np.stack([W_out[k * F:(k + 1) * F] for k in range(4)])
    W2Tk = np.stack([W_out[k * F:(k + 1) * F].T for k in range(4)])
    a2col = np.stack([a2s, a2d], 1)
    return {
        "xT": xT,
        "Wb": W.astype(bf), "WTb": W.transpose(0, 2, 1).astype(bf),
        "acol": acol.astype(bf),
        "W2b": W2k.astype(bf), "W2Tb": W2Tk.astype(bf),
        "a2col": a2col.astype(bf),
        "gidx": cfg["gidx_t"][c], "O8": cfg["O8"][c], "OT8": cfg["OT8"][c],
        "dlb": cfg["dlb"][c],
        "iotab": np.broadcast_to(np.arange(P, dtype=np.float32),
                                 (P, P)).astype(bf).copy(),
        "ident": np.eye(P, dtype=bf),
        "b1rep": np.broadcast_to(bias[:, None, :], (H, P, F)).astype(np.float32).copy(),
        "b2rep": np.broadcast_to(bias2[None, :], (P, C)).astype(np.float32).copy(),
    }


def kernel(**inputs):
    edge = np.asarray(inputs["edge"])
    cfg = preprocess(edge)
    has_bias = bool(np.any(np.asarray(inputs["bias"])))
    has_bias2 = bool(np.any(np.asarray(inputs["bias_out"])))
    key = (cfg["NB"], tuple(cfg["tob"]), has_bias, has_bias2)
    if key not in _CACHE:
        _CACHE[key] = build(cfg, has_bias, has_bias2)
    nc = _CACHE[key]
    in_maps = [_inputs_for_core(cfg, c, inputs) for c in range(NCORES)]
    res = run_bass_kernel_spmd(nc, in_maps, core_ids=list(range(NCORES)))
    outs = [res.results[c]["out"] for c in range(NCORES)]
    full = np.concatenate(outs, 0)[:N]
    return full.astype(np.float32)


if __name__ == "__main__":
    sys.path.insert(0, os.path.dirname(os.path.abspath(__file__)))
    import jax
    with jax.default_device(jax.devices("cpu")[0]):
        import reference
        inputs = {k: np.asarray(v) for k, v in reference.setup_inputs().items()}
        expected = np.asarray(reference.reference(**inputs))
    got = kernel(**inputs)
    rel = np.linalg.norm(got - expected) / np.linalg.norm(expected)
    print("Relative error:", rel)

